# revision 1
# baseline (speedup 1.0000x reference)
"""Trainium2 Bass kernel for a 2-layer LSTM agent (T=1024, B=512, D=H=128).

Strategy:
  - Data-parallel: batch 512 sharded 8 ways -> 64 per core, SPMD one program.
  - State kept TRANSPOSED in SBUF: h/c as [H=128 partitions, B_local free].
    All weights are staged pre-transposed from the host, so no on-device
    transposes are needed anywhere.
  - Matmuls run in bf16 (single-pass PE, fast weight load); PSUM accumulates
    fp32. c-state and activations stay fp32; h is produced bf16 for the
    next matmul.
  - Layer-0 input projection (x @ W_ih0.T) is batched over time: one big
    N=512 matmul per gate per 8-step chunk into a 4-bank PSUM supertile;
    per-step recurrent parts accumulate into 64-wide column slices.
  - Gate biases are pre-added into PSUM with tiny rank-1/rank-4 matmuls
    (lhsT = bias rows, rhs = ones/selector), which lets one ScalarE
    sigmoid cover the i,f,o gates of a layer in a single instruction.
  - c/h updates on VectorE. Heads (actor/critic) are one [128,3].T matmul
    per chunk over the h1 chunk buffer; the tiny +b_head runs on host.
  - x is fed pre-transposed as xT [128, T*64] (host prep) so every DMA is
    contiguous per partition; y returns as yT [3, T*64], rearranged on host.
"""

import sys
import types

if "/opt/trn_rl_repo" not in sys.path:
    sys.path.insert(0, "/opt/trn_rl_repo")

import numpy as np

T, B, D, H = 1024, 512, 128, 128
NCORES = 8
BL = B // NCORES            # 64 batch per core
TC = 8                      # timesteps per chunk
G4 = 4 * H                  # 512
CB = TC * BL                # 512 columns per chunk

# device gate order: i, f, o, g  (PyTorch order is i, f, g, o)
_PERM = np.r_[0:H, H:2 * H, 3 * H:4 * H, 2 * H:3 * H]

_CACHE = {}
HC = 4                      # timesteps per gx half-chunk
HB = HC * BL                # 256 cols per gate per half-chunk


def _install_ntff_shim():
    """Register the axon NTFF profile hook (missing antenv.axon_hooks)."""
    if "antenv.axon_hooks" in sys.modules:
        return
    try:
        from trn_agent_boot.trn_boot import _ntff_profile_via_ctypes
        hook = _ntff_profile_via_ctypes("/opt/axon/libaxon_pjrt.so")
    except Exception:
        hook = None
    m = types.ModuleType("antenv.axon_hooks")
    m.get_axon_ntff_profile_hook = lambda: hook
    sys.modules["antenv.axon_hooks"] = m


def build_program_v4(t_steps=T, mmdt="bf16"):
    import concourse.mybir as mybir
    import concourse.tile as tile
    from concourse import bacc

    f32 = mybir.dt.float32
    DT = {"bf16": mybir.dt.bfloat16, "f32": f32}[mmdt]
    Sig = mybir.ActivationFunctionType.Sigmoid
    Tanh = mybir.ActivationFunctionType.Tanh
    nhalf = t_steps // HC

    nc = bacc.Bacc("TRN2", target_bir_lowering=False, debug=False)

    xT = nc.dram_tensor("xT", (H, t_steps * BL), DT, kind="ExternalInput").ap()
    w0i = nc.dram_tensor("w0i", (H, G4), DT, kind="ExternalInput").ap()
    w0h = nc.dram_tensor("w0h", (H, G4), DT, kind="ExternalInput").ap()
    w1i = nc.dram_tensor("w1i", (H, G4), DT, kind="ExternalInput").ap()
    w1h = nc.dram_tensor("w1h", (H, G4), DT, kind="ExternalInput").ap()
    b0r = nc.dram_tensor("b0r", (1, G4), DT, kind="ExternalInput").ap()
    b1r = nc.dram_tensor("b1r", (4, H), DT, kind="ExternalInput").ap()
    sel1 = nc.dram_tensor("sel1", (4, 4 * BL), DT, kind="ExternalInput").ap()
    whead = nc.dram_tensor("whead", (H, 3), DT, kind="ExternalInput").ap()
    yT = nc.dram_tensor("yT", (3, t_steps * BL), f32, kind="ExternalOutput").ap()

    with tile.TileContext(nc) as tc:
        with (
            tc.tile_pool(name="w", bufs=1) as wp,
            tc.tile_pool(name="x", bufs=3) as xp,
            tc.tile_pool(name="h1", bufs=2) as h1p,
            tc.tile_pool(name="s", bufs=3) as sp,
            tc.tile_pool(name="c", bufs=1) as cp,
            tc.tile_pool(name="ysb", bufs=2) as ysbp,
            tc.tile_pool(name="pgx", bufs=2, space="PSUM") as pgx,
            tc.tile_pool(name="pp1", bufs=2, space="PSUM") as pp1,
            tc.tile_pool(name="ppy", bufs=1, space="PSUM") as ppy,
        ):
            tl_ = {}
            for nm, src, sh in (("w0i", w0i, [H, G4]), ("w0h", w0h, [H, G4]),
                                ("w1i", w1i, [H, G4]), ("w1h", w1h, [H, G4]),
                                ("b0r", b0r, [1, G4]), ("b1r", b1r, [4, H]),
                                ("sel1", sel1, [4, 4 * BL]),
                                ("wh", whead, [H, 3])):
                t_ = wp.tile(sh, DT, tag=nm, name=nm)
                nc.sync.dma_start(t_[:], src)
                tl_[nm] = t_
            ones_t = wp.tile([1, HB], DT, tag="ones")
            nc.vector.memset(ones_t[:], 1.0)

            c0 = cp.tile([H, BL], f32, tag="c0")
            nc.vector.memset(c0[:], 0.0)
            c1 = cp.tile([H, BL], f32, tag="c1")
            nc.vector.memset(c1[:], 0.0)
            z = wp.tile([H, BL], DT, tag="z")
            nc.vector.memset(z[:], 0.0)

            h0 = z
            h1_prev = z[:]

            def emit_half_mms(gxt, xtile):
                """x-projection + bias matmuls (N=128 pieces) for one half."""
                jobs = []
                for g in range(4):
                    for k in range(2):
                        def jx(g=g, k=k):
                            nc.tensor.matmul(
                                gxt[:, g * HB + k * 128:g * HB + k * 128 + 128],
                                lhsT=tl_["w0i"][:, g * H:(g + 1) * H],
                                rhs=xtile[:, k * 128:(k + 1) * 128],
                                start=(g in (0, 2) and k == 0), stop=False,
                                skip_group_check=True)
                        jobs.append(jx)
                for g in range(4):
                    for k in range(2):
                        def jb(g=g, k=k):
                            nc.tensor.matmul(
                                gxt[:, g * HB + k * 128:g * HB + k * 128 + 128],
                                lhsT=tl_["b0r"][0:1, g * H:(g + 1) * H],
                                rhs=ones_t[:, k * 128:(k + 1) * 128],
                                start=False, stop=False, skip_group_check=True)
                        jobs.append(jb)
                return jobs

            def load_x(hf):
                xt = xp.tile([H, HB], DT, tag="xt")
                nc.sync.dma_start(xt[:], xT[:, hf * HB:(hf + 1) * HB])
                return xt

            gx_cur = pgx.tile([H, 4 * HB], f32, tag="gx", name="gx")
            for j in emit_half_mms(gx_cur, load_x(0)):
                j()

            pend_jobs = []
            gx_next = None
            h1t = None
            h1t_old = None
            p1_prev = None
            CBF = 8 * BL

            def make_head_job(h1tile, ch):
                def jh():
                    yps = ppy.tile([3, CBF], f32, tag="yp", name="yp")
                    nc.tensor.matmul(yps[:], lhsT=tl_["wh"][:], rhs=h1tile[:],
                                     start=True, stop=True)
                    ysb = ysbp.tile([3, CBF], f32, tag="ysb", name="ysb")
                    nc.vector.tensor_copy(ysb[:], yps[:])
                    nc.sync.dma_start(yT[:, ch * CBF:(ch + 1) * CBF], ysb[:])
                return jh

            ntau = t_steps + 1
            for tau in range(ntau):
                has_l0 = tau < t_steps
                has_l1 = tau >= 1
                hf, sl = tau // HC, (tau % HC) * BL

                if has_l0 and tau % HC == 0:
                    if gx_next is not None:
                        gx_cur = gx_next
                    if hf + 1 < nhalf:
                        gx_next = pgx.tile([H, 4 * HB], f32, tag="gx",
                                           name="gx")
                        pend_jobs += emit_half_mms(gx_next, load_x(hf + 1))
                    else:
                        gx_next = None
                if has_l0 and tau % 8 == 0:
                    h1t_old, h1t = h1t, h1p.tile([H, CBF], DT, tag="h1t",
                                                 name="h1t")

                # ---- PE: L0 recurrent (g-gate first so tanh starts early) ----
                if has_l0:
                    for g in (3, 0, 1, 2):
                        nc.tensor.matmul(
                            gx_cur[:, g * HB + sl:g * HB + sl + BL],
                            lhsT=tl_["w0h"][:, g * H:(g + 1) * H], rhs=h0[:],
                            start=False, stop=True, skip_group_check=True)
                # ---- PE: L1 for tau-1 ----
                if has_l1:
                    p1 = pp1.tile([H, 4 * BL], f32, tag="p1", name="p1")
                    nc.tensor.matmul(p1[:], lhsT=tl_["b1r"][:],
                                     rhs=tl_["sel1"][:], start=True,
                                     stop=False, skip_group_check=True)
                    for g in range(4):
                        nc.tensor.matmul(
                            p1[:, g * BL:(g + 1) * BL],
                            lhsT=tl_["w1i"][:, g * H:(g + 1) * H], rhs=h0[:],
                            start=False, stop=False, skip_group_check=True)
                    for g in range(4):
                        nc.tensor.matmul(
                            p1[:, g * BL:(g + 1) * BL],
                            lhsT=tl_["w1h"][:, g * H:(g + 1) * H], rhs=h1_prev,
                            start=False, stop=True, skip_group_check=True)
                    p1_prev = p1
                for _ in range(5):
                    if pend_jobs:
                        pend_jobs.pop(0)()

                # ---- L0 chain (critical cycle) ----
                if has_l0:
                    tg0 = sp.tile([H, BL], f32, tag="tg0")
                    nc.scalar.activation(
                        tg0[:], gx_cur[:, 3 * HB + sl:3 * HB + sl + BL], Tanh)
                    sfo0 = sp.tile([H, 3 * BL], f32, tag="sfo0")
                    nc.scalar.activation(
                        sfo0[:].rearrange("p (g x) -> p g x", g=3),
                        gx_cur[:].rearrange("p (g x) -> p g x", g=4)[
                            :, 0:3, sl:sl + BL], Sig)
                    u0 = sp.tile([H, BL], f32, tag="u0")
                    nc.vector.tensor_mul(u0[:], sfo0[:, 0:BL], tg0[:])
                    nc.vector.tensor_mul(c0[:], sfo0[:, BL:2 * BL], c0[:])
                    nc.vector.tensor_add(c0[:], c0[:], u0[:])
                if has_l0:
                    t0_ = sp.tile([H, BL], f32, tag="t0")
                    nc.scalar.activation(t0_[:], c0[:], Tanh)
                    h0n = sp.tile([H, BL], DT, tag="h0")
                    nc.vector.tensor_mul(h0n[:], sfo0[:, 2 * BL:3 * BL],
                                         t0_[:])
                # ---- L1 activations (off-cycle, lower priority) ----
                if has_l1:
                    sfo1 = sp.tile([H, 3 * BL], f32, tag="sfo1")
                    nc.scalar.activation(sfo1[:], p1_prev[:, 0:3 * BL], Sig)
                    tg1 = sp.tile([H, BL], f32, tag="tg1")
                    nc.scalar.activation(tg1[:], p1_prev[:, 3 * BL:4 * BL],
                                         Tanh)
                    u1 = sp.tile([H, BL], f32, tag="u1")
                    nc.vector.tensor_mul(u1[:], sfo1[:, 0:BL], tg1[:])
                    nc.vector.tensor_mul(c1[:], sfo1[:, BL:2 * BL], c1[:])
                    nc.vector.tensor_add(c1[:], c1[:], u1[:])
                    t1_ = sp.tile([H, BL], f32, tag="t1")
                    nc.scalar.activation(t1_[:], c1[:], Tanh)
                    t = tau - 1
                    rotated = has_l0 and tau % 8 == 0
                    dst = h1t_old if rotated else h1t
                    dsl = (t % 8) * BL
                    nc.vector.tensor_mul(dst[:, dsl:dsl + BL],
                                         sfo1[:, 2 * BL:3 * BL], t1_[:])
                    h1_prev = dst[:, dsl:dsl + BL]
                    if (t % 8) == 7:
                        pend_jobs.append(make_head_job(dst, t // 8))
                if has_l0:
                    h0 = h0n

            for j in pend_jobs:
                j()

    nc.compile()
    return nc


def _get_nc(t_steps=T, mmdt="bf16"):
    key = ("nc", t_steps, mmdt)
    if key not in _CACHE:
        _CACHE[key] = build_program_v4(t_steps, mmdt)
    return _CACHE[key]


def _npdt(mmdt):
    if mmdt == "bf16":
        import ml_dtypes
        return ml_dtypes.bfloat16
    return np.float32


def make_in_maps(x, W_ih0, W_hh0, b_ih0, b_hh0, W_ih1, W_hh1, b_ih1, b_hh1,
                 W_actor, b_actor, W_critic, b_critic, t_steps=T, mmdt="bf16"):
    f = np.float32
    dt = _npdt(mmdt)
    w0i = np.ascontiguousarray(np.asarray(W_ih0, f)[_PERM].T).astype(dt)
    w0h = np.ascontiguousarray(np.asarray(W_hh0, f)[_PERM].T).astype(dt)
    w1i = np.ascontiguousarray(np.asarray(W_ih1, f)[_PERM].T).astype(dt)
    w1h = np.ascontiguousarray(np.asarray(W_hh1, f)[_PERM].T).astype(dt)
    b0rw = ((np.asarray(b_ih0, f) + np.asarray(b_hh0, f))[_PERM]
            .reshape(1, G4)).astype(dt)
    b1rw = ((np.asarray(b_ih1, f) + np.asarray(b_hh1, f))[_PERM]
            .reshape(4, H)).astype(dt)
    sel = np.zeros((4, 4 * BL), f)
    for g in range(4):
        sel[g, g * BL:(g + 1) * BL] = 1.0
    sel = sel.astype(dt)
    whead = np.ascontiguousarray(
        np.concatenate([np.asarray(W_actor, f), np.asarray(W_critic, f)], 0).T
    ).astype(dt)
    x = np.asarray(x, f)[:t_steps]
    in_maps = []
    for c in range(NCORES):
        xs = x[:, c * BL:(c + 1) * BL, :]                          # [T, 64, 128]
        xTc = np.ascontiguousarray(
            xs.transpose(2, 0, 1).reshape(H, t_steps * BL)).astype(dt)
        in_maps.append({
            "xT": xTc, "w0i": w0i, "w0h": w0h, "w1i": w1i, "w1h": w1h,
            "b0r": b0rw, "b1r": b1rw, "sel1": sel, "whead": whead,
        })
    return in_maps


def postprocess(results, b_actor, b_critic, t_steps=T):
    bhead = np.concatenate(
        [np.asarray(b_actor, np.float32), np.asarray(b_critic, np.float32)])
    y = np.empty((t_steps, B, 3), np.float32)
    for c in range(NCORES):
        yTc = results[c]["yT"]                                     # [3, T*64]
        y[:, c * BL:(c + 1) * BL, :] = (
            yTc.reshape(3, t_steps, BL).transpose(1, 2, 0) + bhead)
    return y


def run(nc, in_maps, trace=False, tmpdir=None):
    _install_ntff_shim()
    from concourse import bass_utils
    return bass_utils.run_bass_kernel_spmd(
        nc, in_maps, core_ids=list(range(NCORES)), trace=trace, tmpdir=tmpdir)


def kernel(x, W_ih0, W_hh0, b_ih0, b_hh0, W_ih1, W_hh1, b_ih1, b_hh1,
           W_actor, b_actor, W_critic, b_critic):
    mmdt = "bf16"
    key = ("nc4", T, mmdt)
    if key not in _CACHE:
        _CACHE[key] = build_program_v4(T, mmdt)
    nc = _CACHE[key]
    in_maps = make_in_maps(
        x, W_ih0, W_hh0, b_ih0, b_hh0, W_ih1, W_hh1, b_ih1, b_hh1,
        W_actor, b_actor, W_critic, b_critic, T, mmdt)
    res = run(nc, in_maps)
    return postprocess(res.results, b_actor, b_critic, T)

